# revision 6
# baseline (speedup 1.0000x reference)
"""Trainium2 Bass kernel for BertSelfAttention variant (logsigmoid-fused QK attention).

Reference computation (B=2, S=2048, D=1024, H=16, dh=64):
    q = split_heads(hidden @ Wq + bq)
    k = split_heads(hidden @ Wk + bk)
    k = logsigmoid(q) + q + k
    k = logsigmoid(k)
    scores = -(q @ k^T) / 8          # logsigmoid(x) = -softplus(-x), so with
    probs  = softmax(scores)         # kk := softplus(softplus(-q) - q - k_proj) >= 0
    ctx    = probs @ q               # scores == +(q . kk)/8  (no negation needed)

Sharding: 8 cores = 2 (batch) x 4 (head groups of 4 heads / 256 cols of Wq,Wk).
Each core computes its [2048, 256] slice of the output; host reassembles.

Device layout (per core) is fully transposed to keep every matmul transpose-free:
    qT, kkT  [256(dout), 2048(s)]  from  ht=hidden[b].T  (host-side transpose)
    scoresT[kpos, q] = kkT_head^T @ qT_head     (row-packed pairs, K=64)
    expT = Exp(scoresT / 8)                     (one [128,1024] ACT op per chunk)
    ctx_aug[65, q] = sum_kpos v_aug[kpos,65]^T @ expT[kpos, q]
        where v_aug = [v | 1] -> row 64 accumulates the softmax denominator.
    PE-transpose ctx_aug back to [q, 65]; DVE reciprocal + scale; DMA out.
"""

import math

import numpy as np

B, S, D = 2, 2048, 1024
H, DH = 16, 64
NCORES = 8
HG = 4  # head-group count (tensor parallel)
CPG = (H // HG) * DH  # cols per group = 256
NDT = D // 128  # 8 din tiles
NSC = S // 512  # 4 s-chunks (projection) == 4 q-chunks (attention)
NKC = S // 128  # 16 kpos chunks

USE_F32R = True

_compiled = None
LAST_RESULT = None


def _build():
    from contextlib import ExitStack

    import concourse.bacc as bacc
    import concourse.mybir as mybir
    import concourse.tile as tile

    f32 = mybir.dt.float32
    mmdt = mybir.dt.float32r if USE_F32R else mybir.dt.float32
    AF = mybir.ActivationFunctionType

    def r(ap):
        return ap

    def plain(ap):
        return ap.bitcast(f32) if USE_F32R else ap

    nc = bacc.Bacc("TRN2", target_bir_lowering=False, debug=False)
    ht = nc.dram_tensor("ht", [D, S], mmdt, kind="ExternalInput").ap()
    wq = nc.dram_tensor("wq", [D, CPG], mmdt, kind="ExternalInput").ap()
    wk = nc.dram_tensor("wk", [D, CPG], mmdt, kind="ExternalInput").ap()
    # smalls cols: pbq[0:2] nbq[2:4] nbk[4:6] idA[6:71] idB[71:135] ones[135:151]
    smalls = nc.dram_tensor("smalls", [128, 151], f32, kind="ExternalInput").ap()
    out = nc.dram_tensor("out", [S, CPG], f32, kind="ExternalOutput").ap()

    with tile.TileContext(nc) as tc, ExitStack() as ctx:
        const = ctx.enter_context(tc.tile_pool(name="const", bufs=1))
        big = ctx.enter_context(tc.tile_pool(name="big", bufs=1))
        sb = ctx.enter_context(tc.tile_pool(name="sb", bufs=2))

        sm = const.tile([128, 151], f32, tag="smalls")
        nc.sync.dma_start(sm[:], smalls[:])
        pbq_t = sm[:, 0:2]
        nbq_t = sm[:, 2:4]
        nbk_t = sm[:, 4:6]
        ida_t = sm[:, 6:71]
        idb_t = sm[:, 71:135]
        ones_t = sm[:, 135:151]

        wqs, wks, hts = [], [], []
        for j in range(NDT):
            w = const.tile([128, CPG], mmdt, tag=f"wq{j}", name=f"wqs{j}")
            nc.sync.dma_start(w[:], wq[j * 128 : (j + 1) * 128, :])
            wqs.append(w)
            w = const.tile([128, CPG], mmdt, tag=f"wk{j}", name=f"wks{j}")
            nc.sync.dma_start(w[:], wk[j * 128 : (j + 1) * 128, :])
            wks.append(w)
        for j in range(NDT):
            t_ = big.tile([128, S], mmdt, tag=f"ht{j}", name=f"hts{j}")
            for sc in range(NSC):
                nc.sync.dma_start(
                    t_[:, sc * 512 : (sc + 1) * 512],
                    ht[j * 128 : (j + 1) * 128, sc * 512 : (sc + 1) * 512],
                )
            hts.append(t_)

        q_sb = [big.tile([128, S], mmdt, tag=f"q{t}", name=f"q{t}") for t in range(2)]
        kk_sb = [big.tile([128, S], mmdt, tag=f"kk{t}", name=f"kk{t}") for t in range(2)]
        vaug = [big.tile([128, NKC * 65], mmdt, tag=f"v{h}", name=f"v{h}") for h in range(4)]

        # ---- projection phase: qT/kkT [256, 2048] ----
        with tc.tile_pool(name="pj", bufs=2, space="PSUM") as pj, tc.tile_pool(
            name="tpv", bufs=2, space="PSUM"
        ) as tpvp:
            for t in range(2):
                for sc in range(NSC):
                    ssl = slice(sc * 512, (sc + 1) * 512)
                    qp = pj.tile([128, 512], f32, tag="qp")
                    for j in range(NDT):
                        nc.tensor.matmul(
                            qp[:],
                            lhsT=r(wqs[j][:, t * 128 : (t + 1) * 128]),
                            rhs=r(hts[j][:, ssl]),
                            start=(j == 0),
                            stop=(j == NDT - 1),
                        )
                    kp = pj.tile([128, 512], f32, tag="kp")
                    for j in range(NDT):
                        nc.tensor.matmul(
                            kp[:],
                            lhsT=r(wks[j][:, t * 128 : (t + 1) * 128]),
                            rhs=r(hts[j][:, ssl]),
                            start=(j == 0),
                            stop=(j == NDT - 1),
                        )
                    # softplus(x) = ln(exp(x) + 1): no Softplus table on this
                    # HW, but Exp and Ln share one table (no reload thrash).
                    # t1 = softplus(-(q_raw + bq));  q = q_raw + bq
                    e1 = sb.tile([128, 512], f32, tag="e1")
                    nc.scalar.activation(
                        e1[:], qp[:], AF.Exp, bias=nbq_t[:, t : t + 1], scale=-1.0
                    )
                    t1 = sb.tile([128, 512], f32, tag="t1")
                    nc.scalar.activation(t1[:], e1[:], AF.Ln, bias=1.0, scale=1.0)
                    nc.vector.tensor_scalar_add(q_sb[t][:, ssl], qp[:], pbq_t[:, t : t + 1])
                    # kk = softplus(t1 - q - k_raw - bk)
                    s1 = sb.tile([128, 512], f32, tag="s1")
                    nc.vector.tensor_sub(s1[:], t1[:], plain(q_sb[t][:, ssl]))
                    t2 = sb.tile([128, 512], f32, tag="t2")
                    nc.vector.tensor_sub(t2[:], s1[:], kp[:])
                    e2 = sb.tile([128, 512], f32, tag="e2")
                    nc.scalar.activation(
                        e2[:], t2[:], AF.Exp, bias=nbk_t[:, t : t + 1], scale=1.0
                    )
                    nc.scalar.activation(
                        kk_sb[t][:, ssl], e2[:], AF.Ln, bias=1.0, scale=1.0
                    )

            # ---- v_aug: per-head [s, 64 | 1] via PE transpose of qT ----
            for lh in range(4):
                t, rr = lh // 2, lh % 2
                hsl = slice(rr * 64, rr * 64 + 64)
                vv = vaug[lh][:].rearrange("p (c w) -> p c w", w=65)
                nc.vector.tensor_copy(
                    vv[:, :, 64:65], ones_t.rearrange("p (c w) -> p c w", w=1)
                )
                for j in range(NKC):
                    tpv = tpvp.tile([128, 64], f32, tag="tpv")
                    nc.tensor.transpose(
                        tpv[:],
                        plain(q_sb[t][hsl, j * 128 : (j + 1) * 128]),
                        idb_t[hsl, 0:64],
                    )
                    nc.vector.tensor_copy(vaug[lh][:, j * 65 : j * 65 + 64], tpv[:])

        # ---- attention phase ----
        with tc.tile_pool(name="sp", bufs=2, space="PSUM") as spp, tc.tile_pool(
            name="ctxp", bufs=1, space="PSUM"
        ) as ctxp, tc.tile_pool(name="tpp", bufs=2, space="PSUM") as tpp, tc.tile_pool(
            name="etp", bufs=3
        ) as etp, tc.tile_pool(name="csp", bufs=2) as csp, tc.tile_pool(
            name="recp", bufs=4
        ) as recp, tc.tile_pool(name="osbp", bufs=2) as osbp:
            for qc in range(NSC):
                qsl = slice(qc * 512, (qc + 1) * 512)
                osb = [osbp.tile([128, CPG], f32, tag=f"osb{j2}", name=f"osb{j2}") for j2 in range(4)]
                for t in range(2):
                    ctxs = [
                        ctxp.tile([65, 512], f32, tag="ctxA", name="ctxA"),
                        ctxp.tile([65, 512], f32, tag="ctxB", name="ctxB"),
                    ]
                    for kc in range(NKC):
                        ksl = slice(kc * 128, (kc + 1) * 128)
                        sp = spp.tile([128, 1024], f32, tag="sp")
                        nc.tensor.matmul(
                            sp[:, 0:512],
                            lhsT=r(kk_sb[t][0:64, ksl]),
                            rhs=r(q_sb[t][0:64, qsl]),
                            start=True,
                            stop=True,
                        )
                        nc.tensor.matmul(
                            sp[:, 512:1024],
                            lhsT=r(kk_sb[t][64:128, ksl]),
                            rhs=r(q_sb[t][64:128, qsl]),
                            start=True,
                            stop=True,
                        )
                        et = etp.tile([128, 1024], mmdt, tag="et")
                        nc.scalar.activation(et[:], sp[:], AF.Exp, scale=0.125)
                        for rr in range(2):
                            nc.tensor.matmul(
                                ctxs[rr][:],
                                lhsT=r(vaug[2 * t + rr][:, kc * 65 : kc * 65 + 65]),
                                rhs=r(et[:, rr * 512 : rr * 512 + 512]),
                                start=(kc == 0),
                                stop=(kc == NKC - 1),
                            )
                    for rr in range(2):
                        lh = 2 * t + rr
                        cs = csp.tile([128, 512], f32, tag="cs")
                        nc.vector.tensor_copy(cs[0:65, :], ctxs[rr][:])
                        for j2 in range(4):
                            tp = tpp.tile([128, 65], f32, tag="tp")
                            nc.tensor.transpose(
                                tp[:], cs[0:65, j2 * 128 : (j2 + 1) * 128], ida_t[0:65, 0:65]
                            )
                            rec = recp.tile([128, 1], f32, tag="rec")
                            nc.vector.reciprocal(rec[:], tp[:, 64:65])
                            nc.vector.tensor_scalar_mul(
                                osb[j2][:, lh * 64 : lh * 64 + 64], tp[:, 0:64], rec[:]
                            )
                for j2 in range(4):
                    qt = qc * 4 + j2
                    nc.sync.dma_start(out[qt * 128 : (qt + 1) * 128, :], osb[j2][:])

    nc.compile()
    return nc


def kernel(hidden_states, attention_mask, Wq, bq, Wk, bk):
    global _compiled, LAST_RESULT
    hs = np.asarray(hidden_states, dtype=np.float32)
    am = np.asarray(attention_mask)
    Wq = np.asarray(Wq, dtype=np.float32)
    Wk = np.asarray(Wk, dtype=np.float32)
    bq = np.asarray(bq, dtype=np.float32)
    bk = np.asarray(bk, dtype=np.float32)

    if _compiled is None:
        _compiled = _build()
    nc = _compiled

    from concourse.bass_utils import run_bass_kernel_spmd

    def to_mmdt(x):
        # fp32r = 1s/8e/11m (top 20 bits of fp32); round-to-nearest-even so the
        # values we hand the PE are exactly representable.
        if not USE_F32R:
            return np.ascontiguousarray(x, dtype=np.float32)
        b = np.ascontiguousarray(x, dtype=np.float32).view(np.uint32)
        lsb = (b >> np.uint32(12)) & np.uint32(1)
        r = (b + np.uint32(0x7FF) + lsb) & np.uint32(0xFFFFF000)
        return r.view(np.float32)

    ida = np.eye(128, dtype=np.float32)[:, :65]
    idb = np.tile(np.eye(64, dtype=np.float32), (2, 1))
    in_maps = []
    for c in range(NCORES):
        b, g = c // HG, c % HG
        cols = slice(g * CPG, (g + 1) * CPG)
        bq_s = bq[cols].reshape(2, 128).T
        bk_s = bk[cols].reshape(2, 128).T
        smalls = np.concatenate(
            [bq_s, -bq_s, -bk_s, ida, idb, np.ones((128, 16), np.float32)], axis=1
        ).astype(np.float32)
        in_maps.append(
            {
                "ht": to_mmdt(hs[b].T),
                "wq": to_mmdt(Wq[:, cols]),
                "wk": to_mmdt(Wk[:, cols]),
                "smalls": np.ascontiguousarray(smalls),
            }
        )

    res = run_bass_kernel_spmd(nc, in_maps, list(range(NCORES)))
    LAST_RESULT = res

    outp = np.empty((B, S, H * DH), dtype=np.float32)
    for c in range(NCORES):
        b, g = c // HG, c % HG
        outp[b, :, g * CPG : (g + 1) * CPG] = res.results[c]["out"]

    # attention_mask==0 masks whole query rows -> uniform probs -> ctx row is
    # the mean of q over all key positions. Never triggers for all-ones masks.
    if (am == 0).any():
        for b in range(B):
            rows = np.nonzero(am[b] == 0)[0]
            if rows.size:
                q_full = hs[b] @ Wq + bq
                outp[b, rows, :] = q_full.mean(axis=0)
    return outp


# revision 8
# speedup vs baseline: 1.3891x; 1.3891x over previous
"""Trainium2 Bass kernel for BertSelfAttention variant (logsigmoid-fused QK attention).

Reference computation (B=2, S=2048, D=1024, H=16, dh=64):
    q = split_heads(hidden @ Wq + bq)
    k = split_heads(hidden @ Wk + bk)
    k1 = logsigmoid(q) + q + k
    k2 = logsigmoid(k1)
    scores = -(q @ k2^T) / 8
    probs  = softmax(scores)
    ctx    = probs @ q

With kk := -k2 = softplus(softplus(-q) - q - k) >= 0, scores == +(q . kk)/8.
Exact identity (avoids a Softplus table, which this HW lacks):
    kk = ln(1 + e^{-q-k} + e^{-2q-k})
All Exp ops run first and the Ln runs once per half in a batched pass, so the
ACT engine uses one activation table at a time (no per-op table reloads).

Sharding: 8 cores = 2 (batch) x 4 (head groups of 4 heads / 256 cols of Wq,Wk).
Each core computes its [2048, 256] slice of the output; host reassembles.

Device layout (per core) is fully transposed to keep every matmul transpose-free:
    qT, kkT  [256(dout), 2048(s)]  from  ht=hidden[b].T  (host-side transpose)
    scoresT[kpos, q] = kkT_head^T @ qT_head     (row-packed head pairs, K=64)
    expT = Exp(scoresT / 8)                     (one [128,1024] ACT op per chunk)
    ctx_aug[65, q] = sum_kpos v_aug[kpos,65]^T @ expT[kpos, q]
        where v_aug = [v | 1] -> row 64 accumulates the softmax denominator.
    PE-transpose ctx_aug back to [q, 65]; DVE reciprocal + scale; DMA out.

Matmuls run in bf16 (measured end-to-end absmax rel err ~6e-3); softmax
normalization, biases, and the output path stay fp32.
"""

import numpy as np

B, S, D = 2, 2048, 1024
H, DH = 16, 64
NCORES = 8
HG = 4  # head-group count (tensor parallel)
CPG = (H // HG) * DH  # cols per group = 256
NDT = D // 128  # 8 din tiles
NSC = S // 512  # 4 s-chunks (projection) == 4 q-chunks (attention)
NKC = S // 128  # 16 kpos chunks

MM_DTYPE = "bf16"  # "bf16" | "f32r" | "f32"

_compiled = None
LAST_RESULT = None


def _build():
    from contextlib import ExitStack

    import concourse.bacc as bacc
    import concourse.mybir as mybir
    import concourse.tile as tile

    f32 = mybir.dt.float32
    mmdt = {
        "bf16": mybir.dt.bfloat16,
        "f32r": mybir.dt.float32r,
        "f32": mybir.dt.float32,
    }[MM_DTYPE]
    AF = mybir.ActivationFunctionType

    nc = bacc.Bacc("TRN2", target_bir_lowering=False, debug=False)
    ht = nc.dram_tensor("ht", [D, S], mmdt, kind="ExternalInput").ap()
    wq = nc.dram_tensor("wq", [D, CPG], mmdt, kind="ExternalInput").ap()
    wk = nc.dram_tensor("wk", [D, CPG], mmdt, kind="ExternalInput").ap()
    # smalls cols: pbq[0:2] nbqk[2:4] nb2qk[4:6] idA[6:71] ones[71:87]
    smalls = nc.dram_tensor("smalls", [128, 87], f32, kind="ExternalInput").ap()
    # identity for the v transposes, in the matmul dtype (I64 stacked twice)
    idb = nc.dram_tensor("idb", [128, 64], mmdt, kind="ExternalInput").ap()
    out = nc.dram_tensor("out", [S, CPG], f32, kind="ExternalOutput").ap()

    with tile.TileContext(nc) as tc, ExitStack() as ctx:
        const = ctx.enter_context(tc.tile_pool(name="const", bufs=1))
        big = ctx.enter_context(tc.tile_pool(name="big", bufs=1))
        sb = ctx.enter_context(tc.tile_pool(name="sb", bufs=2))

        sm = const.tile([128, 87], f32, tag="smalls")
        nc.sync.dma_start(sm[:], smalls[:])
        pbq_t = sm[:, 0:2]
        nbqk_t = sm[:, 2:4]
        nb2qk_t = sm[:, 4:6]
        ida_t = sm[:, 6:71]
        ones_t = sm[:, 71:87]
        idb_t = const.tile([128, 64], mmdt, tag="idb")
        nc.sync.dma_start(idb_t[:], idb[:])

        wqs, wks, hts = [], [], []
        for j in range(NDT):
            w = const.tile([128, CPG], mmdt, tag=f"wq{j}", name=f"wqs{j}")
            nc.sync.dma_start(w[:], wq[j * 128 : (j + 1) * 128, :])
            wqs.append(w)
            w = const.tile([128, CPG], mmdt, tag=f"wk{j}", name=f"wks{j}")
            nc.sync.dma_start(w[:], wk[j * 128 : (j + 1) * 128, :])
            wks.append(w)
        for j in range(NDT):
            t_ = big.tile([128, S], mmdt, tag=f"ht{j}", name=f"hts{j}")
            for sc in range(NSC):
                nc.sync.dma_start(
                    t_[:, sc * 512 : (sc + 1) * 512],
                    ht[j * 128 : (j + 1) * 128, sc * 512 : (sc + 1) * 512],
                )
            hts.append(t_)

        q_sb = [big.tile([128, S], mmdt, tag=f"q{t}", name=f"q{t}") for t in range(2)]
        kk_sb = [big.tile([128, S], mmdt, tag=f"kk{t}", name=f"kk{t}") for t in range(2)]
        kst = [
            big.tile([128, S], f32, tag=f"kst{t}", name=f"kst{t}") for t in range(2)
        ]
        vaug = [
            big.tile([128, NKC * 65], mmdt, tag=f"v{h}", name=f"v{h}") for h in range(4)
        ]

        # ---- projection phase: qT [256,2048] (bf16) and kk staging (f32) ----
        with tc.tile_pool(name="pj", bufs=2, space="PSUM") as pj, tc.tile_pool(
            name="tpv", bufs=2, space="PSUM"
        ) as tpvp:
            for t in range(2):
                for sc in range(NSC):
                    ssl = slice(sc * 512, (sc + 1) * 512)
                    qp = pj.tile([128, 512], f32, tag="qp")
                    for j in range(NDT):
                        nc.tensor.matmul(
                            qp[:],
                            lhsT=wqs[j][:, t * 128 : (t + 1) * 128],
                            rhs=hts[j][:, ssl],
                            start=(j == 0),
                            stop=(j == NDT - 1),
                        )
                    kp = pj.tile([128, 512], f32, tag="kp")
                    for j in range(NDT):
                        nc.tensor.matmul(
                            kp[:],
                            lhsT=wks[j][:, t * 128 : (t + 1) * 128],
                            rhs=hts[j][:, ssl],
                            start=(j == 0),
                            stop=(j == NDT - 1),
                        )
                    # kk = ln(1 + e^{-q-k} + e^{-2q-k}) with q = qp+bq, k = kp+bk:
                    #   a  = qp + kp   (k evicted first: only one PSUM read/op)
                    #   eu = Exp(-a - (bq+bk))
                    #   bb = a + qp
                    #   ev = Exp(-bb - (2bq+bk))
                    #   kst = eu + ev          (Ln(kst + 1) batched later)
                    kc_ = sb.tile([128, 512], f32, tag="kc_")
                    nc.vector.tensor_copy(kc_[:], kp[:])
                    a = sb.tile([128, 512], f32, tag="a")
                    nc.vector.tensor_add(a[:], kc_[:], qp[:])
                    eu = sb.tile([128, 512], f32, tag="eu")
                    nc.scalar.activation(
                        eu[:], a[:], AF.Exp, bias=nbqk_t[:, t : t + 1], scale=-1.0
                    )
                    bb = sb.tile([128, 512], f32, tag="bb")
                    nc.vector.tensor_add(bb[:], a[:], qp[:])
                    ev = sb.tile([128, 512], f32, tag="ev")
                    nc.scalar.activation(
                        ev[:], bb[:], AF.Exp, bias=nb2qk_t[:, t : t + 1], scale=-1.0
                    )
                    nc.vector.tensor_add(kst[t][:, ssl], eu[:], ev[:])
                    nc.vector.tensor_scalar_add(
                        q_sb[t][:, ssl], qp[:], pbq_t[:, t : t + 1]
                    )
            # batched Ln: one table switch for the whole kernel tail
            for t in range(2):
                nc.scalar.activation(kk_sb[t][:], kst[t][:], AF.Ln, bias=1.0, scale=1.0)

            # ---- v_aug: per-head [s, 64 | 1] via PE transpose of qT ----
            for lh in range(4):
                t, rr = lh // 2, lh % 2
                hsl = slice(rr * 64, rr * 64 + 64)
                vv = vaug[lh][:].rearrange("p (c w) -> p c w", w=65)
                nc.vector.tensor_copy(
                    vv[:, :, 64:65], ones_t.rearrange("p (c w) -> p c w", w=1)
                )
                for j in range(NKC):
                    tpv = tpvp.tile([128, 64], mmdt, tag="tpv")
                    nc.tensor.transpose(
                        tpv[:],
                        q_sb[t][hsl, j * 128 : (j + 1) * 128],
                        idb_t[hsl, 0:64],
                    )
                    nc.vector.tensor_copy(vaug[lh][:, j * 65 : j * 65 + 64], tpv[:])

        # ---- attention phase ----
        with tc.tile_pool(name="sp", bufs=2, space="PSUM") as spp, tc.tile_pool(
            name="ctxp", bufs=1, space="PSUM"
        ) as ctxp, tc.tile_pool(name="tpp", bufs=2, space="PSUM") as tpp, tc.tile_pool(
            name="etp", bufs=3
        ) as etp, tc.tile_pool(name="csp", bufs=2) as csp, tc.tile_pool(
            name="recp", bufs=4
        ) as recp, tc.tile_pool(name="osbp", bufs=2) as osbp:
            for qc in range(NSC):
                qsl = slice(qc * 512, (qc + 1) * 512)
                osb = [
                    osbp.tile([128, CPG], f32, tag=f"osb{j2}", name=f"osb{j2}")
                    for j2 in range(4)
                ]
                for t in range(2):
                    ctxs = [
                        ctxp.tile([65, 512], f32, tag="ctxA", name="ctxA"),
                        ctxp.tile([65, 512], f32, tag="ctxB", name="ctxB"),
                    ]
                    for kc in range(NKC):
                        ksl = slice(kc * 128, (kc + 1) * 128)
                        sp = spp.tile([128, 1024], f32, tag="sp")
                        nc.tensor.matmul(
                            sp[:, 0:512],
                            lhsT=kk_sb[t][0:64, ksl],
                            rhs=q_sb[t][0:64, qsl],
                            start=True,
                            stop=True,
                        )
                        nc.tensor.matmul(
                            sp[:, 512:1024],
                            lhsT=kk_sb[t][64:128, ksl],
                            rhs=q_sb[t][64:128, qsl],
                            start=True,
                            stop=True,
                        )
                        et = etp.tile([128, 1024], mmdt, tag="et")
                        nc.scalar.activation(et[:], sp[:], AF.Exp, scale=0.125)
                        for rr in range(2):
                            nc.tensor.matmul(
                                ctxs[rr][:],
                                lhsT=vaug[2 * t + rr][:, kc * 65 : kc * 65 + 65],
                                rhs=et[:, rr * 512 : rr * 512 + 512],
                                start=(kc == 0),
                                stop=(kc == NKC - 1),
                            )
                    for rr in range(2):
                        lh = 2 * t + rr
                        cs = csp.tile([128, 512], f32, tag="cs")
                        nc.vector.tensor_copy(cs[0:65, :], ctxs[rr][:])
                        for j2 in range(4):
                            tp = tpp.tile([128, 65], f32, tag="tp")
                            nc.tensor.transpose(
                                tp[:],
                                cs[0:65, j2 * 128 : (j2 + 1) * 128],
                                ida_t[0:65, 0:65],
                            )
                            rec = recp.tile([128, 1], f32, tag="rec")
                            nc.vector.reciprocal(rec[:], tp[:, 64:65])
                            nc.vector.tensor_scalar_mul(
                                osb[j2][:, lh * 64 : lh * 64 + 64], tp[:, 0:64], rec[:]
                            )
                for j2 in range(4):
                    qt = qc * 4 + j2
                    nc.sync.dma_start(out[qt * 128 : (qt + 1) * 128, :], osb[j2][:])

    nc.compile()
    return nc


def kernel(hidden_states, attention_mask, Wq, bq, Wk, bk):
    global _compiled, LAST_RESULT
    hs = np.asarray(hidden_states, dtype=np.float32)
    am = np.asarray(attention_mask)
    Wq = np.asarray(Wq, dtype=np.float32)
    Wk = np.asarray(Wk, dtype=np.float32)
    bq = np.asarray(bq, dtype=np.float32)
    bk = np.asarray(bk, dtype=np.float32)

    if _compiled is None:
        _compiled = _build()
    nc = _compiled

    from concourse.bass_utils import run_bass_kernel_spmd

    if MM_DTYPE == "bf16":
        import ml_dtypes

        def to_mmdt(x):
            return np.ascontiguousarray(
                np.asarray(x, np.float32).astype(ml_dtypes.bfloat16)
            )

    elif MM_DTYPE == "f32r":

        def to_mmdt(x):
            # fp32r = 1s/8e/11m (top 20 bits of fp32), round-to-nearest-even
            b = np.ascontiguousarray(x, dtype=np.float32).view(np.uint32)
            lsb = (b >> np.uint32(12)) & np.uint32(1)
            r = (b + np.uint32(0x7FF) + lsb) & np.uint32(0xFFFFF000)
            return r.view(np.float32)

    else:

        def to_mmdt(x):
            return np.ascontiguousarray(x, dtype=np.float32)

    ida = np.eye(128, dtype=np.float32)[:, :65]
    idb = to_mmdt(np.tile(np.eye(64, dtype=np.float32), (2, 1)))
    in_maps = []
    for c in range(NCORES):
        b, g = c // HG, c % HG
        cols = slice(g * CPG, (g + 1) * CPG)
        bq_s = bq[cols].reshape(2, 128).T
        bk_s = bk[cols].reshape(2, 128).T
        smalls = np.concatenate(
            [bq_s, -(bq_s + bk_s), -(2 * bq_s + bk_s), ida, np.ones((128, 16), np.float32)],
            axis=1,
        ).astype(np.float32)
        in_maps.append(
            {
                "ht": to_mmdt(hs[b].T),
                "wq": to_mmdt(Wq[:, cols]),
                "wk": to_mmdt(Wk[:, cols]),
                "smalls": np.ascontiguousarray(smalls),
                "idb": idb,
            }
        )

    res = run_bass_kernel_spmd(nc, in_maps, list(range(NCORES)))
    LAST_RESULT = res

    outp = np.empty((B, S, H * DH), dtype=np.float32)
    for c in range(NCORES):
        b, g = c // HG, c % HG
        outp[b, :, g * CPG : (g + 1) * CPG] = res.results[c]["out"]

    # attention_mask==0 masks whole query rows -> uniform probs -> ctx row is
    # the mean of q over all key positions. Never triggers for all-ones masks.
    if (am == 0).any():
        for b in range(B):
            rows = np.nonzero(am[b] == 0)[0]
            if rows.size:
                q_full = hs[b] @ Wq + bq
                outp[b, rows, :] = q_full.mean(axis=0)
    return outp


# revision 10
# speedup vs baseline: 1.4503x; 1.0441x over previous
"""Trainium2 Bass kernel for BertSelfAttention variant (logsigmoid-fused QK attention).

Reference computation (B=2, S=2048, D=1024, H=16, dh=64):
    q = split_heads(hidden @ Wq + bq)
    k = split_heads(hidden @ Wk + bk)
    k1 = logsigmoid(q) + q + k
    k2 = logsigmoid(k1)
    scores = -(q @ k2^T) / 8
    probs  = softmax(scores)
    ctx    = probs @ q

With kk := -k2 = softplus(softplus(-q) - q - k) >= 0, scores == +(q . kk)/8.
Exact identity (avoids a Softplus table, which this HW lacks):
    kk = ln(1 + e^{-q-k} + e^{-2q-k})
All Exp ops run first and the Ln runs once per half in a batched pass, so the
ACT engine uses one activation table at a time (no per-op table reloads).

Sharding: 8 cores = 2 (batch) x 4 (head groups of 4 heads / 256 cols of Wq,Wk).
Each core computes its [2048, 256] slice of the output; host reassembles.

Device layout (per core) is fully transposed to keep every matmul transpose-free:
    qT, kkT  [256(dout), 2048(s)]  from  ht=hidden[b].T  (host-side transpose)
    scoresT[kpos, q] = kkT_head^T @ qT_head     (row-packed head pairs, K=64)
    expT = Exp(scoresT / 8)                     (one [128,1024] ACT op per chunk)
    ctx_aug[65, q] = sum_kpos v_aug[kpos,65]^T @ expT[kpos, q]
        where v_aug = [v | 1] -> row 64 accumulates the softmax denominator.
    PE-transpose ctx_aug back to [q, 65]; DVE reciprocal + scale; DMA out.

Matmuls run in bf16 (measured end-to-end absmax rel err ~6e-3); softmax
normalization, biases, and the output path stay fp32.
"""

import numpy as np

B, S, D = 2, 2048, 1024
H, DH = 16, 64
NCORES = 8
HG = 4  # head-group count (tensor parallel)
CPG = (H // HG) * DH  # cols per group = 256
NDT = D // 128  # 8 din tiles
NSC = S // 512  # 4 s-chunks (projection) == 4 q-chunks (attention)
NKC = S // 128  # 16 kpos chunks

MM_DTYPE = "bf16"  # "bf16" | "f32r" | "f32"

_compiled = None
LAST_RESULT = None


def _build():
    from contextlib import ExitStack

    import concourse.bacc as bacc
    import concourse.mybir as mybir
    import concourse.tile as tile

    f32 = mybir.dt.float32
    mmdt = {
        "bf16": mybir.dt.bfloat16,
        "f32r": mybir.dt.float32r,
        "f32": mybir.dt.float32,
    }[MM_DTYPE]
    AF = mybir.ActivationFunctionType

    nc = bacc.Bacc("TRN2", target_bir_lowering=False, debug=False)
    ht = nc.dram_tensor("ht", [D, S], mmdt, kind="ExternalInput").ap()
    wq = nc.dram_tensor("wq", [D, CPG], mmdt, kind="ExternalInput").ap()
    wk = nc.dram_tensor("wk", [D, CPG], mmdt, kind="ExternalInput").ap()
    # smalls cols: pbq[0:2] nbqk[2:4] nb2qk[4:6] idA[6:71] ones[71:87]
    smalls = nc.dram_tensor("smalls", [128, 87], f32, kind="ExternalInput").ap()
    # identity for the v transposes, in the matmul dtype (I64 stacked twice)
    idb = nc.dram_tensor("idb", [128, 64], mmdt, kind="ExternalInput").ap()
    out = nc.dram_tensor("out", [S, CPG], f32, kind="ExternalOutput").ap()

    with tile.TileContext(nc) as tc, ExitStack() as ctx:
        const = ctx.enter_context(tc.tile_pool(name="const", bufs=1))
        big = ctx.enter_context(tc.tile_pool(name="big", bufs=1))
        sb = ctx.enter_context(tc.tile_pool(name="sb", bufs=2))

        sm = const.tile([128, 87], f32, tag="smalls")
        nc.sync.dma_start(sm[:], smalls[:])
        pbq_t = sm[:, 0:2]
        nbqk_t = sm[:, 2:4]
        nb2qk_t = sm[:, 4:6]
        ida_t = sm[:, 6:71]
        ones_t = sm[:, 71:87]
        idb_t = const.tile([128, 64], mmdt, tag="idb")
        nc.sync.dma_start(idb_t[:], idb[:])

        wqs, wks, hts = [], [], []
        for j in range(NDT):
            w = const.tile([128, CPG], mmdt, tag=f"wq{j}", name=f"wqs{j}")
            nc.sync.dma_start(w[:], wq[j * 128 : (j + 1) * 128, :])
            wqs.append(w)
            w = const.tile([128, CPG], mmdt, tag=f"wk{j}", name=f"wks{j}")
            nc.sync.dma_start(w[:], wk[j * 128 : (j + 1) * 128, :])
            wks.append(w)
        for j in range(NDT):
            t_ = big.tile([128, S], mmdt, tag=f"ht{j}", name=f"hts{j}")
            for sc in range(NSC):
                nc.sync.dma_start(
                    t_[:, sc * 512 : (sc + 1) * 512],
                    ht[j * 128 : (j + 1) * 128, sc * 512 : (sc + 1) * 512],
                )
            hts.append(t_)

        q_sb = [big.tile([128, S], mmdt, tag=f"q{t}", name=f"q{t}") for t in range(2)]
        kk_sb = [big.tile([128, S], mmdt, tag=f"kk{t}", name=f"kk{t}") for t in range(2)]
        kst = [
            big.tile([128, S], f32, tag=f"kst{t}", name=f"kst{t}") for t in range(2)
        ]
        vaug = [
            big.tile([128, NKC * 65], mmdt, tag=f"v{h}", name=f"v{h}") for h in range(4)
        ]

        # ---- projection phase: qT [256,2048] (bf16) and kk staging (f32) ----
        with tc.tile_pool(name="pj", bufs=2, space="PSUM") as pj, tc.tile_pool(
            name="tpv", bufs=2, space="PSUM"
        ) as tpvp:
            for t in range(2):
                for sc in range(NSC):
                    ssl = slice(sc * 512, (sc + 1) * 512)
                    qp = pj.tile([128, 512], f32, tag="qp")
                    for j in range(NDT):
                        nc.tensor.matmul(
                            qp[:],
                            lhsT=wqs[j][:, t * 128 : (t + 1) * 128],
                            rhs=hts[j][:, ssl],
                            start=(j == 0),
                            stop=(j == NDT - 1),
                        )
                    kp = pj.tile([128, 512], f32, tag="kp")
                    for j in range(NDT):
                        nc.tensor.matmul(
                            kp[:],
                            lhsT=wks[j][:, t * 128 : (t + 1) * 128],
                            rhs=hts[j][:, ssl],
                            start=(j == 0),
                            stop=(j == NDT - 1),
                        )
                    # kk = ln(1 + e^{-q-k} + e^{-2q-k}) with q = qp+bq, k = kp+bk:
                    #   a  = qp + kp   (k evicted first: only one PSUM read/op)
                    #   eu = Exp(-a - (bq+bk))
                    #   bb = a + qp
                    #   ev = Exp(-bb - (2bq+bk))
                    #   kst = eu + ev          (Ln(kst + 1) batched later)
                    kc_ = sb.tile([128, 512], f32, tag="kc_")
                    nc.vector.tensor_copy(kc_[:], kp[:])
                    a = sb.tile([128, 512], f32, tag="a")
                    nc.vector.tensor_add(a[:], kc_[:], qp[:])
                    eu = sb.tile([128, 512], f32, tag="eu")
                    nc.scalar.activation(
                        eu[:], a[:], AF.Exp, bias=nbqk_t[:, t : t + 1], scale=-1.0
                    )
                    bb = sb.tile([128, 512], f32, tag="bb")
                    nc.vector.tensor_add(bb[:], a[:], qp[:])
                    ev = sb.tile([128, 512], f32, tag="ev")
                    nc.scalar.activation(
                        ev[:], bb[:], AF.Exp, bias=nb2qk_t[:, t : t + 1], scale=-1.0
                    )
                    nc.vector.tensor_add(kst[t][:, ssl], eu[:], ev[:])
                    nc.vector.tensor_scalar_add(
                        q_sb[t][:, ssl], qp[:], pbq_t[:, t : t + 1]
                    )
            # batched Ln: one table switch for the whole kernel tail
            for t in range(2):
                nc.scalar.activation(kk_sb[t][:], kst[t][:], AF.Ln, bias=1.0, scale=1.0)

            # ---- v_aug: per-head [s, 64 | 1] via PE transpose of qT ----
            for lh in range(4):
                t, rr = lh // 2, lh % 2
                hsl = slice(rr * 64, rr * 64 + 64)
                vv = vaug[lh][:].rearrange("p (c w) -> p c w", w=65)
                nc.vector.tensor_copy(
                    vv[:, :, 64:65], ones_t.rearrange("p (c w) -> p c w", w=1)
                )
                for j in range(NKC):
                    tpv = tpvp.tile([128, 64], mmdt, tag="tpv")
                    nc.tensor.transpose(
                        tpv[:],
                        q_sb[t][hsl, j * 128 : (j + 1) * 128],
                        idb_t[hsl, 0:64],
                    )
                    nc.vector.tensor_copy(vaug[lh][:, j * 65 : j * 65 + 64], tpv[:])

        # ---- attention phase ----
        with tc.tile_pool(name="sp", bufs=2, space="PSUM") as spp, tc.tile_pool(
            name="ctxp", bufs=1, space="PSUM"
        ) as ctxp, tc.tile_pool(name="tpp", bufs=2, space="PSUM") as tpp, tc.tile_pool(
            name="etp", bufs=20
        ) as etp, tc.tile_pool(name="csp", bufs=2) as csp, tc.tile_pool(
            name="recp", bufs=4
        ) as recp, tc.tile_pool(name="osbp", bufs=2) as osbp:
            for qc in range(NSC):
                qsl = slice(qc * 512, (qc + 1) * 512)
                osb = [
                    osbp.tile([128, CPG], f32, tag=f"osb{j2}", name=f"osb{j2}")
                    for j2 in range(4)
                ]
                for t in range(2):
                    # stream: scores + exp for all kpos chunks (et stored)
                    ets = []
                    for kc in range(NKC):
                        ksl = slice(kc * 128, (kc + 1) * 128)
                        sp = spp.tile([128, 1024], f32, tag="sp")
                        nc.tensor.matmul(
                            sp[:, 0:512],
                            lhsT=kk_sb[t][0:64, ksl],
                            rhs=q_sb[t][0:64, qsl],
                            start=True,
                            stop=True,
                        )
                        nc.tensor.matmul(
                            sp[:, 512:1024],
                            lhsT=kk_sb[t][64:128, ksl],
                            rhs=q_sb[t][64:128, qsl],
                            start=True,
                            stop=True,
                        )
                        et = etp.tile([128, 1024], mmdt, tag="et", name=f"et{kc}")
                        nc.scalar.activation(et[:], sp[:], AF.Exp, scale=0.125)
                        ets.append(et)
                    # drain: ctx matmuls back-to-back (waits already satisfied)
                    ctxs = [
                        ctxp.tile([65, 512], f32, tag="ctxA", name="ctxA"),
                        ctxp.tile([65, 512], f32, tag="ctxB", name="ctxB"),
                    ]
                    for kc in range(NKC):
                        for rr in range(2):
                            nc.tensor.matmul(
                                ctxs[rr][:],
                                lhsT=vaug[2 * t + rr][:, kc * 65 : kc * 65 + 65],
                                rhs=ets[kc][:, rr * 512 : rr * 512 + 512],
                                start=(kc == 0),
                                stop=(kc == NKC - 1),
                            )
                    for rr in range(2):
                        lh = 2 * t + rr
                        cs = csp.tile([128, 512], f32, tag="cs")
                        nc.vector.tensor_copy(cs[0:65, :], ctxs[rr][:])
                        for j2 in range(4):
                            tp = tpp.tile([128, 65], f32, tag="tp")
                            nc.tensor.transpose(
                                tp[:],
                                cs[0:65, j2 * 128 : (j2 + 1) * 128],
                                ida_t[0:65, 0:65],
                            )
                            rec = recp.tile([128, 1], f32, tag="rec")
                            nc.vector.reciprocal(rec[:], tp[:, 64:65])
                            nc.vector.tensor_scalar_mul(
                                osb[j2][:, lh * 64 : lh * 64 + 64], tp[:, 0:64], rec[:]
                            )
                for j2 in range(4):
                    qt = qc * 4 + j2
                    nc.sync.dma_start(out[qt * 128 : (qt + 1) * 128, :], osb[j2][:])

    nc.compile()
    return nc


def kernel(hidden_states, attention_mask, Wq, bq, Wk, bk):
    global _compiled, LAST_RESULT
    hs = np.asarray(hidden_states, dtype=np.float32)
    am = np.asarray(attention_mask)
    Wq = np.asarray(Wq, dtype=np.float32)
    Wk = np.asarray(Wk, dtype=np.float32)
    bq = np.asarray(bq, dtype=np.float32)
    bk = np.asarray(bk, dtype=np.float32)

    if _compiled is None:
        _compiled = _build()
    nc = _compiled

    from concourse.bass_utils import run_bass_kernel_spmd

    if MM_DTYPE == "bf16":
        import ml_dtypes

        def to_mmdt(x):
            return np.ascontiguousarray(
                np.asarray(x, np.float32).astype(ml_dtypes.bfloat16)
            )

    elif MM_DTYPE == "f32r":

        def to_mmdt(x):
            # fp32r = 1s/8e/11m (top 20 bits of fp32), round-to-nearest-even
            b = np.ascontiguousarray(x, dtype=np.float32).view(np.uint32)
            lsb = (b >> np.uint32(12)) & np.uint32(1)
            r = (b + np.uint32(0x7FF) + lsb) & np.uint32(0xFFFFF000)
            return r.view(np.float32)

    else:

        def to_mmdt(x):
            return np.ascontiguousarray(x, dtype=np.float32)

    ida = np.eye(128, dtype=np.float32)[:, :65]
    idb = to_mmdt(np.tile(np.eye(64, dtype=np.float32), (2, 1)))
    in_maps = []
    for c in range(NCORES):
        b, g = c // HG, c % HG
        cols = slice(g * CPG, (g + 1) * CPG)
        bq_s = bq[cols].reshape(2, 128).T
        bk_s = bk[cols].reshape(2, 128).T
        smalls = np.concatenate(
            [bq_s, -(bq_s + bk_s), -(2 * bq_s + bk_s), ida, np.ones((128, 16), np.float32)],
            axis=1,
        ).astype(np.float32)
        in_maps.append(
            {
                "ht": to_mmdt(hs[b].T),
                "wq": to_mmdt(Wq[:, cols]),
                "wk": to_mmdt(Wk[:, cols]),
                "smalls": np.ascontiguousarray(smalls),
                "idb": idb,
            }
        )

    res = run_bass_kernel_spmd(nc, in_maps, list(range(NCORES)))
    LAST_RESULT = res

    outp = np.empty((B, S, H * DH), dtype=np.float32)
    for c in range(NCORES):
        b, g = c // HG, c % HG
        outp[b, :, g * CPG : (g + 1) * CPG] = res.results[c]["out"]

    # attention_mask==0 masks whole query rows -> uniform probs -> ctx row is
    # the mean of q over all key positions. Never triggers for all-ones masks.
    if (am == 0).any():
        for b in range(B):
            rows = np.nonzero(am[b] == 0)[0]
            if rows.size:
                q_full = hs[b] @ Wq + bq
                outp[b, rows, :] = q_full.mean(axis=0)
    return outp
